# revision 40
# baseline (speedup 1.0000x reference)
"""SSD-style CustomLoss (Huber loc loss + hard-negative-mined CE conf loss)
as a Trainium2 Bass/Tile kernel, data-parallel over the batch axis on 8
NeuronCores.

v4 design (vs. baseline):
  - bf16 inputs (halves HBM traffic; label one-hots and nonzero-ness of
    deltas are preserved exactly by the cast).
  - CE-from-probs collapses via the one-hot identity to
    cp = ln(clip(<y,x>/sum_c x, eps, 1-eps)) -- no per-class normalize/
    clip/log tensors.
  - Partition layout p = (image, 16 sub-rows); tokens stream in 4 chunks
    of 2208 tokens x 8 images. All heavy elementwise work runs on
    [128, 2898]-shaped tiles (big instructions, few bubbles).
  - Per-token sums over C=21 via bf16 tensor_tensor trees (2x DVE mode)
    instead of 1x tensor_reduce.
  - Huber in exact relu^2 form: hub = 0.5 d^2 - 0.5 relu(d-1)^2
    - 0.5 min(d+1, 0)^2, with Square+accum on the scalar engine.
  - Hard-negative threshold via 12-round bisection; per-image counts with
    ONE tensor_scalar+accum per round thanks to the image-major partition
    layout, cross-partition block-sum via a blockdiag ones matmul.
"""

import os
import sys
import types

import numpy as np

import concourse.bass as bass
import concourse.mybir as mybir
from concourse.bass_utils import run_bass_kernel_spmd
from concourse.mybir import ActivationFunctionType as Act
from concourse.mybir import AluOpType as Op
from concourse.tile import TileContext, add_dep_helper

B, N, C = 64, 8732, 21
NCORES = 8
NIMG = B // NCORES          # images per core
SUBS = 16                   # sub-rows per image -> 8*16 = 128 partitions
TPS = 552                   # tokens per sub-row: 16*552 = 8832 >= 8732
NPAD = SUBS * TPS           # padded tokens per image
NCHUNK = 4
TPC = TPS // NCHUNK         # tokens per partition per chunk = 138
CP = 24                     # classes padded to a power-of-two-ish tree width;
                            # pad x=0 (exp adds 3 to s1, corrected via Ln bias)
FC = TPC * CP               # label elems per partition per chunk
FD = TPC * 8                # 1104 delta elems per partition per chunk
EPS = 1e-7
BIG = 1.0e30
PLPAD = -20.0               # pl value for padded tokens
T_BISECT = 11
BISECT_LO = 1.0
BISECT_W0 = 8.0             # interval [lo, lo+2w); resolution 16/2^11
F32 = mybir.dt.float32
BF16 = mybir.dt.bfloat16
NPBF16 = np.dtype(mybir.dt.np(mybir.dt.bfloat16))
X = mybir.AxisListType.X

LAST_RESULTS = None

# The walrus build in this container rejects instructions carrying more than
# MAX_WAITS semaphore waits. Tile's scheduler freely emits more, so split the
# excess onto NoOps inserted just before the offending instruction.
MAX_WAITS = 1
NOP_WAITS = 1


def _ensure_ntff_hook():
    """bass_utils' axon trace path imports antenv.axon_hooks, which this
    image lacks. Synthesize the tiny get/set registry and install the
    ctypes-based NTFF hook the boot would have registered."""
    try:
        from antenv.axon_hooks import get_axon_ntff_profile_hook  # noqa: F401

        return
    except ImportError:
        pass
    try:
        import antenv
        from trn_agent_boot.trn_boot import _ntff_profile_via_ctypes

        m = types.ModuleType("antenv.axon_hooks")
        _reg = [None]
        m.set_axon_ntff_profile_hook = lambda h: _reg.__setitem__(0, h)
        m.get_axon_ntff_profile_hook = lambda: _reg[0]
        sys.modules["antenv.axon_hooks"] = m
        antenv.axon_hooks = m
        m.set_axon_ntff_profile_hook(
            _ntff_profile_via_ctypes("/opt/axon/libaxon_pjrt.so")
        )
    except Exception:
        pass


def _split_excess_waits(bir_json: bytes) -> bytes:
    import json as _json

    m = _json.loads(bir_json)
    ctr = 0
    for fdef in m["functions"]:
        for blk in fdef["blocks"]:
            insts = blk["instructions"]
            out = []
            for ins in insts:
                si = ins.get("sync_info")
                ow = (si or {}).get("on_wait") or []
                cap = NOP_WAITS if ins.get("opcode") in ("NoOp", "Drain") else MAX_WAITS
                if len(ow) > cap:
                    keep = ow[-cap:]
                    excess = ow[:-cap]
                    si["on_wait"] = keep
                    while excess:
                        chunk, excess = excess[:NOP_WAITS], excess[NOP_WAITS:]
                        ctr += 1
                        out.append(
                            {
                                "debug": ins.get("debug"),
                                "engine": ins["engine"],
                                "ins": [],
                                "name": f"I-wsplit-{ctr}",
                                "opcode": "NoOp",
                                "outs": [],
                                "sync_info": {"on_update": [], "on_wait": chunk},
                            }
                        )
                out.append(ins)
            blk["instructions"] = out
    return _json.dumps(m).encode()


def _patch_wait_splitting(nc):
    orig = nc.to_json_bytes

    def patched():
        return _split_excess_waits(orig())

    nc.to_json_bytes = patched
    return nc


def _col(ap3, j):
    """[128, T, W] view -> [128, T] view of column j (stride W)."""
    return ap3[:, :, j : j + 1].rearrange("p t o -> p (t o)")


def emit_program(nc, xl, al, dl, w16, ones, out):
    from contextlib import ExitStack

    with TileContext(nc) as tc, ExitStack() as stk:
        per = stk.enter_context(tc.tile_pool(name="per", bufs=1))
        cp_pool = stk.enter_context(tc.tile_pool(name="chunk", bufs=3))
        pp = stk.enter_context(tc.tile_pool(name="ps", bufs=2, space="PSUM"))

        # persistent per-core state
        mrm = per.tile([128, TPS], F32)     # masked ranking values (mr)
        cpt = per.tile([128, TPS], F32)     # ln(clip(x_k / s2)) per token
        posm = per.tile([128, TPS], BF16)   # positive mask
        sall = per.tile([128, TPS, 3], F32)  # (s1, ax, s2) per token
        pc = per.tile([128, NCHUNK], F32)   # per-partition pos counts by chunk
        d2s = per.tile([128, NCHUNK], F32)  # sum d^2*pos by chunk
        r1s = per.tile([128, NCHUNK], F32)  # sum relu(d-1)^2*pos
        r2s = per.tile([128, NCHUNK], F32)  # sum min(d+1,0)^2*pos
        w16t = per.tile([128, 128], F32)
        onest = per.tile([128, 128], F32)

        for k in range(NCHUNK):
            ksl = slice(k * TPC, (k + 1) * TPC)
            xt = cp_pool.tile([128, FC], BF16, tag="xt")
            at = cp_pool.tile([128, FC], BF16, tag="at")
            dt = cp_pool.tile([128, FD], BF16, tag="dt")
            if k == 0:
                h = FC // 2
                nc.sync.dma_start(xt[:, 0:h], xl[k][:, 0:h])
                nc.sync.dma_start(xt[:, h:FC], xl[k][:, h:FC])
            else:
                nc.sync.dma_start(xt[:], xl[k])
            nc.sync.dma_start(at[:], al[k])
            nc.sync.dma_start(dt[:], dl[k])

            x3 = xt[:].rearrange("p (t c) -> p t c", c=CP)
            a3 = at[:].rearrange("p (t c) -> p t c", c=CP)
            d3 = dt[:].rearrange("p (t c) -> p t c", c=8)

            # --- CE stats: s1 = sum exp x (+3 from class pads), ax = <y,x>,
            # s2 = sum x. Sum over CP=24 via a tensor_tensor add tree whose
            # lower levels are merged across the three stats. ---
            et = cp_pool.tile([128, FC], BF16, tag="et")
            if k == 0:
                h = FC // 2
                nc.scalar.activation(et[:, 0:h], xt[:, 0:h], Act.Exp)
                nc.scalar.activation(et[:, h:FC], xt[:, h:FC], Act.Exp)
            else:
                nc.scalar.activation(et[:], xt[:], Act.Exp)
            axt = cp_pool.tile([128, FC], BF16, tag="axt")
            nc.vector.tensor_mul(axt[:], at[:], xt[:])

            e3 = et[:].rearrange("p (t c) -> p t c", c=CP)
            ax3 = axt[:].rearrange("p (t c) -> p t c", c=CP)
            t12 = cp_pool.tile([128, TPC, 3, 12], BF16, tag="t12")
            for si, src3 in enumerate((e3, ax3, x3)):
                nc.vector.tensor_tensor(
                    t12[:, :, si, :], src3[:, :, 0:12], src3[:, :, 12:24],
                    op=Op.add,
                )
            t6 = cp_pool.tile([128, TPC, 3, 6], BF16, tag="t6")
            nc.vector.tensor_tensor(
                t6[:], t12[:, :, :, 0:6], t12[:, :, :, 6:12], op=Op.add
            )
            t3 = cp_pool.tile([128, TPC, 3, 3], BF16, tag="t3")
            nc.vector.tensor_tensor(
                t3[:], t6[:, :, :, 0:3], t6[:, :, :, 3:6], op=Op.add
            )
            t3a = t3[:, :, :, 0:1].rearrange("p t s o -> p t (s o)")
            t3b = t3[:, :, :, 1:2].rearrange("p t s o -> p t (s o)")
            t3c = t3[:, :, :, 2:3].rearrange("p t s o -> p t (s o)")
            sA = cp_pool.tile([128, TPC, 3], F32, tag="sA")
            nc.vector.tensor_tensor(sA[:], t3a, t3b, op=Op.add)
            nc.vector.tensor_tensor(sall[:, ksl, :], sA[:], t3c, op=Op.add)

            # --- positives: any |actual delta| > 0 (via sum of squares) ---
            sq = cp_pool.tile([128, TPC, 4], BF16, tag="sq")
            nc.scalar.activation(
                sq[:], d3[:, :, 4:8], Act.Square
            )
            p1 = cp_pool.tile([128, TPC, 2], BF16, tag="p1")
            nc.gpsimd.tensor_tensor(p1[:], sq[:, :, 0:2], sq[:, :, 2:4], op=Op.add)
            s4 = cp_pool.tile([128, TPC], F32, tag="s4")
            nc.gpsimd.tensor_tensor(s4[:], _col(p1, 0), _col(p1, 1), op=Op.add)
            nc.vector.tensor_scalar(
                posm[:, ksl], s4[:], 0.0, 0.0, Op.is_gt, Op.add,
                accum_out=pc[:, k : k + 1],
            )

            # --- Huber: hub = 0.5 d^2 - 0.5 relu(d-1)^2 - 0.5 min(d+1,0)^2,
            #     with d pre-masked by pos so negatives contribute 0 ---
            dd = cp_pool.tile([128, TPC, 4], BF16, tag="dd")
            nc.gpsimd.tensor_sub(dd[:], d3[:, :, 0:4], d3[:, :, 4:8])
            dpos = cp_pool.tile([128, TPC, 4], BF16, tag="dpos")
            pos_b = posm[:, ksl][:, :, None].broadcast_to([128, TPC, 4])
            nc.gpsimd.tensor_tensor(dpos[:], dd[:], pos_b, op=Op.mult)

            dsq = cp_pool.tile([128, TPC, 4], BF16, tag="dsq")
            nc.scalar.activation(
                dsq[:], dpos[:], Act.Square, accum_out=d2s[:, k : k + 1]
            )
            re1 = cp_pool.tile([128, TPC, 4], BF16, tag="re1")
            nc.vector.tensor_scalar(re1[:], dpos[:], 1.0, 0.0, Op.subtract, Op.max)
            rsq = cp_pool.tile([128, TPC, 4], BF16, tag="rsq")
            nc.scalar.activation(
                rsq[:], re1[:], Act.Square, accum_out=r1s[:, k : k + 1]
            )
            re2 = cp_pool.tile([128, TPC, 4], BF16, tag="re2")
            nc.vector.tensor_scalar(re2[:], dpos[:], 1.0, 0.0, Op.add, Op.min)
            msq = cp_pool.tile([128, TPC, 4], BF16, tag="msq")
            nc.scalar.activation(
                msq[:], re2[:], Act.Square, accum_out=r2s[:, k : k + 1]
            )

        # ---- weights for cross-partition sums (needed from here on) ----
        nc.sync.dma_start(w16t[:], w16[:])
        nc.sync.dma_start(onest[:], ones[:])

        s1v = sall[:, :, 0:1].rearrange("p t o -> p (t o)")
        axv = sall[:, :, 1:2].rearrange("p t o -> p (t o)")
        s2v = sall[:, :, 2:3].rearrange("p t o -> p (t o)")

        # ---- batched ranking values: mr = ln(s1 - 3) - ax (the -3 removes
        # the three exp(0)=1 class pads), minus BIG at pos ----
        s1c = per.tile([128, TPS], F32)
        nc.vector.tensor_scalar_add(s1c[:], s1v, -3.0)
        lse = per.tile([128, TPS], F32)
        nc.scalar.activation(lse[:], s1c[:], Act.Ln)
        mrv = per.tile([128, TPS], F32)
        nc.vector.tensor_sub(mrv[:], lse[:], axv)
        negm = per.tile([128, TPS], BF16)
        nc.vector.tensor_scalar(negm[:], posm[:], -BIG, None, Op.mult)
        nc.vector.tensor_add(mrm[:], mrv[:], negm[:])

        # ---- per-image positive counts, broadcast within image blocks ----
        pcv = per.tile([128, 1], F32)
        nc.vector.reduce_sum(pcv[:], pc[:], axis=X)
        pcb = pp.tile([128, 1], F32)
        nc.tensor.matmul(pcb[:], w16t[:], pcv[:], start=True, stop=True)
        kimg = per.tile([128, 1], F32)
        nc.vector.tensor_scalar(kimg[:], pcb[:], 3.0, None, Op.mult)

        # hub + pos totals don't depend on the bisection: reduce and ship
        # them now so only the small conf sum trails the threshold search.
        htmp = per.tile([128, NCHUNK], F32)
        nc.vector.tensor_sub(htmp[:], d2s[:], r1s[:])
        nc.vector.tensor_sub(htmp[:], htmp[:], r2s[:])
        hsum = per.tile([128, 1], F32)
        nc.vector.reduce_sum(hsum[:], htmp[:], axis=X)
        pk = per.tile([128, 2], F32)
        nc.vector.tensor_copy(pk[:, 0:1], hsum[:])
        nc.vector.tensor_copy(pk[:, 1:2], pcv[:])
        pkr = pp.tile([128, 2], F32)
        nc.tensor.matmul(pkr[:], onest[:], pk[:], start=True, stop=True)
        outhp = per.tile([1, 2], F32)
        nc.vector.tensor_copy(outhp[:], pkr[0:1, :])
        nc.sync.dma_start(out[:, 0:2], outhp[:])

        # ---- bisection for per-image rank-k threshold on mrm ----
        lo = per.tile([128, 1], F32)
        nc.vector.memset(lo[:], BISECT_LO)
        mid = per.tile([128, 1], F32)
        cmpd = per.tile([128, TPS], F32)
        cnt = per.tile([128, 1], F32)
        ge = per.tile([128, 1], mybir.dt.int32)
        # cp = ln(clip(ax/s2, eps, 1-eps)) computed division-free as
        # clip(0.5*(ln ax^2 - ln s2^2), ln eps, ln(1-eps)), patched to
        # ln(eps) where sign(ax) != sign(s2). The DVE pieces are
        # interleaved into the bisection loop (runs in PE round-trip slack);
        # the four Act ops go to the otherwise-idle scalar engine.
        LNEPS = float(np.log(EPS))
        LN1ME = float(np.log1p(-EPS))
        a2t = per.tile([128, TPS], F32)
        b2t = per.tile([128, TPS], F32)
        sgn = per.tile([128, TPS], mybir.dt.int32)
        lnepst = per.tile([128, TPS], F32)
        nc.scalar.activation(a2t[:], axv, Act.Square)
        nc.scalar.activation(b2t[:], s2v, Act.Square)
        nc.scalar.activation(a2t[:], a2t[:], Act.Ln)
        nc.scalar.activation(b2t[:], b2t[:], Act.Ln)

        scr2 = lse  # dead after mrm; scratch for accumulations

        # cp numerator-denominator difference on gpsimd (idle during the
        # bisection); cheap tensor_scalar pieces interleave on DVE below.
        nc.gpsimd.tensor_sub(a2t[:], a2t[:], b2t[:])      # a2t = ln ax^2 - ln s2^2
        nc.gpsimd.tensor_mul(b2t[:], axv, s2v)            # b2t = ax * s2 (sign)

        w = BISECT_W0
        for _t in range(T_BISECT):
            wmid = w
            nc.vector.tensor_scalar_add(mid[:], lo[:], wmid)
            w *= 0.5
            nc.vector.tensor_scalar(
                cmpd[:], mrm[:], mid[:], 0.0, Op.is_ge, Op.add,
                accum_out=cnt[:],
            )
            cps = pp.tile([128, 1], F32, tag="cps")
            nc.tensor.matmul(cps[:], w16t[:], cnt[:], start=True, stop=True)
            if _t == 2:
                nc.vector.memset(lnepst[:], LNEPS)
            elif _t == 4:
                nc.vector.tensor_scalar(
                    cpt[:], a2t[:], 0.5, LNEPS, Op.mult, Op.max
                )
            elif _t == 5:
                nc.vector.tensor_scalar_min(cpt[:], cpt[:], LN1ME)
            elif _t == 6:
                nc.vector.tensor_scalar(sgn[:], b2t[:], 0.0, None, Op.is_lt)
            nc.vector.tensor_tensor(ge[:], cps[:], kimg[:], op=Op.is_ge)
            nc.vector.copy_predicated(lo[:], ge[:], mid[:])
        nc.vector.copy_predicated(cpt[:], sgn[:], lnepst[:])

        # ---- final masked conf sum (only remaining bisect-dependent value) ----
        selv = per.tile([128, TPS], BF16)
        nc.vector.tensor_scalar(selv[:], mrm[:], lo[:], None, Op.is_ge)
        sel2 = per.tile([128, TPS], BF16)
        nc.vector.tensor_tensor(sel2[:], selv[:], posm[:], op=Op.add)
        scr = mrv  # dead after mrm is built; reuse as scratch
        nc.vector.tensor_mul(scr[:], cpt[:], sel2[:])
        csum = per.tile([128, 1], F32)
        nc.vector.tensor_scalar(
            scr2[:], scr[:], 0.0, 0.0, Op.add, Op.add, accum_out=csum[:]
        )
        csr = pp.tile([128, 1], F32)
        nc.tensor.matmul(csr[:], onest[:], csum[:], start=True, stop=True)
        outt = per.tile([1, 1], F32)
        i_cp = nc.vector.tensor_copy(outt[:], csr[0:1, :])
        i_dma = nc.sync.dma_start(out[:, 2:3], outt[:])

        n1 = nc.sync.nop()
        add_dep_helper(n1.ins, i_cp.ins, sync=True, reason="funnel-dve")
        n2 = nc.sync.nop()
        add_dep_helper(n2.ins, i_dma.ins, sync=True, reason="funnel-dma")

    return nc


def build_bass():
    nc = bass.Bass()
    xl = nc.dram_tensor("xl", [NCHUNK, 128, FC], BF16, kind="ExternalInput")
    al = nc.dram_tensor("al", [NCHUNK, 128, FC], BF16, kind="ExternalInput")
    dl = nc.dram_tensor("dl", [NCHUNK, 128, FD], BF16, kind="ExternalInput")
    w16 = nc.dram_tensor("w16", [128, 128], F32, kind="ExternalInput")
    ones = nc.dram_tensor("ones", [128, 128], F32, kind="ExternalInput")
    out = nc.dram_tensor("out", [1, 4], F32, kind="ExternalOutput")
    emit_program(nc, xl, al, dl, w16, ones, out)
    return _patch_wait_splitting(nc)


def _to_chunks(x, fill, dpad=0):
    """[NIMG, N, D] f32 -> [NCHUNK, 128, TPC*(D+dpad)] bf16 in the
    p=(image,sub) / token-chunk layout. Class pads (dpad) are zero."""
    nimg, n, dd = x.shape
    buf = np.zeros((nimg, NPAD, dd + dpad), dtype=np.float32)
    buf[:, :n, :dd] = x
    buf[:, n:, :dd] = fill
    # token T = s*TPS + k*TPC + pos
    buf = buf.reshape(nimg, SUBS, NCHUNK, TPC, dd + dpad)
    buf = buf.transpose(2, 0, 1, 3, 4)  # (k, i, s, pos, d)
    buf = buf.reshape(NCHUNK, 128, TPC * (dd + dpad))
    return np.ascontiguousarray(buf.astype(NPBF16))


def kernel(actual_bbox_deltas, actual_labels, pred_bbox_deltas, pred_labels):
    global LAST_RESULTS
    ab = np.asarray(actual_bbox_deltas, dtype=np.float32)
    al_ = np.asarray(actual_labels, dtype=np.float32)
    pb = np.asarray(pred_bbox_deltas, dtype=np.float32)
    pl_ = np.asarray(pred_labels, dtype=np.float32)
    assert pl_.shape == (B, N, C), pl_.shape

    # deltas interleaved per token: (pd0..3, ad0..3)
    pdad = np.concatenate([pb, ab], axis=2)  # [B, N, 8]

    blk = np.arange(128) // SUBS
    w16 = (blk[:, None] == blk[None, :]).astype(np.float32)
    ones = np.ones((128, 128), np.float32)

    nc = build_bass()
    in_maps = []
    for c in range(NCORES):
        sl = slice(c * NIMG, (c + 1) * NIMG)
        in_maps.append(
            {
                "xl": _to_chunks(pl_[sl], PLPAD, dpad=CP - C),
                "al": _to_chunks(al_[sl], 0.0, dpad=CP - C),
                "dl": _to_chunks(pdad[sl], 0.0),
                "w16": w16,
                "ones": ones,
            }
        )

    trace = bool(int(os.environ.get("KERNEL_TRACE", "0")))
    if trace:
        _ensure_ntff_hook()
    res = run_bass_kernel_spmd(
        nc, in_maps, core_ids=list(range(NCORES)), trace=trace
    )
    LAST_RESULTS = res

    hub_sum = 0.0
    cesel_sum = 0.0
    pos_total = 0.0
    for r in res.results:
        o = r["out"].reshape(-1)
        hub_sum += float(o[0])
        pos_total += float(o[1])
        cesel_sum += float(o[2])

    total_pos = max(pos_total, 1.0)
    loc_loss = np.float32(0.25 * 0.5 * hub_sum / total_pos)
    conf_loss = np.float32(-cesel_sum / total_pos)
    return loc_loss, conf_loss
